# revision 2
# baseline (speedup 1.0000x reference)
"""Masked dot-product attention on 8 Trainium2 NeuronCores (valid-rows-only).

Full inputs: queries/keys/values [16, 2048, 128] f32, valid_lens [16] int.
The reference masks whole query rows q >= valid_len (softmax of a constant
row = uniform weights = mean(V)), so only sum(valid_lens) ~ 47% of rows
need real attention.  Masked rows are filled with mean(V) on the host.

Device-side plan (single SPMD program shared by all 8 cores):
  - Host cuts each batch's valid rows into 512-row and 128-row sections
    and deals them to cores so every core runs the same static unit list
    (one 128-unit first for a fast first matmul, then the 512-units, then
    the remaining 128-units; a few padded units).
  - Per-core data is pre-staged by the host in fp16: qT [128d, ROWS],
    and per-unit copies of that unit's K^T [128d, 2048k] and
    [V|1] [128k, 16kt, 129].
  - Per unit: S^T[k,q] = kT . qT on PE into psum chunks; exp of ktiles
    0-11 on ScalarE (exact spline), ktiles 12-15 on DVE via a one-
    instruction Schraudolph approximation (int16(s*A+B) bitcast as fp16,
    mean-centered so the softmax normalization cancels the bias); PE
    computes PV[q,129] = E_chunk^T . [V|1]; DVE divides by the last
    column; fp16 out, descrambled on the host.
  - A dozen dependency-free warmup matmuls on scratch SBUF run during
    the DMA head so the PE p-state is ramped before real work arrives.
"""

import math
from contextlib import ExitStack

import numpy as np

import concourse.bacc as bacc
import concourse.tile as tile
from concourse import mybir
from concourse.bass_utils import run_bass_kernel_spmd

B, Q, K, D = 16, 2048, 2048, 128
NCORES = 8
P = 128
NKT = K // P                 # 16 k-tiles
WBIG = 512                   # big unit width (4 qsubs)
WSM = 128                    # small unit width
SCALE = 1.0 / math.sqrt(D)
SPLIT = 12                   # ktiles 0..SPLIT-1 on ScalarE, rest on DVE

# fp16 Schraudolph: bits = round(s*EXPA + EXPB); bitcast(bits) ~ exp(s*SCALE)
# EXPB is lowered by the mean log2 interpolation error (2 - 1/ln2 - 1/2) so
# the sawtooth error is zero-mean and cancels against the exact-exp ktiles
# in the softmax normalization.
EXPA = 1024.0 / math.log(2.0) * SCALE
EXPB = 1024.0 * (15.0 - (2.0 - 1.0 / math.log(2.0) - 0.5))

F32 = mybir.dt.float32
F16 = mybir.dt.float16
I16 = mybir.dt.int16

NWARM = 12                   # warmup matmuls (128 cols each)


# ----------------------------------------------------------------------------
# host-side planning
# ----------------------------------------------------------------------------

def _sections(vl):
    out = []
    for v in vl:
        a = v // WBIG
        rem = v - a * WBIG
        nb_ = (rem + WSM - 1) // WSM
        if nb_ * WSM >= WBIG:
            a += 1
            nb_ = 0
        out.append((a, nb_))
    return out


def _plan(valid_lens):
    """Returns (na, nbm, cores); cores[c] = unit list (batch, row0, width),
    batch None for padding, ordered [small, big*na, small*(nbm-1)]."""
    vl = [int(v) for v in valid_lens]
    ab = _sections(vl)
    A = sum(a for a, _ in ab)
    Bs = sum(b for _, b in ab)
    best = None
    for na in range((A + NCORES - 1) // NCORES, -1, -1):
        d = max(0, A - NCORES * na)
        btot = Bs + 4 * d
        nbm = (btot + NCORES - 1) // NCORES
        cost = NCORES * (WBIG * na + WSM * nbm)
        if cost >= WBIG * A + WSM * Bs:
            if best is None or cost < best[0]:
                best = (cost, na, nbm, d)
    _, na, nbm, d = best
    ab = list(ab)
    for b in sorted(range(B), key=lambda x: -ab[x][0]):
        if d == 0:
            break
        a, s = ab[b]
        if a > 0:
            ab[b] = (a - 1, s + 4)
            d -= 1
    big_secs, sm_secs = [], []
    for b in range(B):
        a, s = ab[b]
        for i in range(a):
            big_secs.append((b, i * WBIG))
        for i in range(s):
            sm_secs.append((b, a * WBIG + i * WSM))
    cores = []
    bi = si = 0
    for c in range(NCORES):
        bigs, smalls = [], []
        for _ in range(na):
            bigs.append((*big_secs[bi], WBIG) if bi < len(big_secs)
                        else (None, 0, WBIG))
            bi += 1
        for _ in range(nbm):
            smalls.append((*sm_secs[si], WSM) if si < len(sm_secs)
                          else (None, 0, WSM))
            si += 1
        # one small unit first (fast first matmul), then bigs, then the rest
        if smalls:
            cores.append([smalls[0]] + bigs + smalls[1:])
        else:
            cores.append(bigs)
    return na, nbm, cores


def _widths(na, nbm):
    if nbm > 0:
        return [WSM] + [WBIG] * na + [WSM] * (nbm - 1)
    return [WBIG] * na


def _core_arrays(units, keysT16, vb16, q32, vl):
    rows = sum(w for _, _, w in units)
    nu = len(units)
    qT = np.zeros((P, rows), dtype=np.float16)
    kT = np.zeros((P, nu, K), dtype=np.float16)
    vb = np.zeros((P, nu, NKT, D + 1), dtype=np.float16)
    col = 0
    for i, (b, r0, w) in enumerate(units):
        if b is not None:
            kT[:, i, :] = keysT16[b]
            vb[:, i, :, :] = vb16[b]
            nr = max(0, min(w, vl[b] - r0))
            if nr > 0:
                qT[:, col:col + nr] = q32[b, r0:r0 + nr, :].T
        else:
            vb[:, i, :, D] = 1.0      # keep denominators nonzero on padding
        col += w
    return {"qt": qT, "kt": kT, "vb": vb}


# ----------------------------------------------------------------------------
# bass program (shared across cores; depends only on (na, nbm))
# ----------------------------------------------------------------------------

def _chunks_for(w):
    """(n_ktiles, engine) chunk list; 's' = ScalarE exact exp,
    'v' = DVE Schraudolph.  Scalar ktiles must total SPLIT."""
    if w == WBIG:
        return [(3, "s"), (3, "s"), (3, "s"), (3, "s"), (2, "v"), (2, "v")]
    return [(8, "s"), (4, "s"), (4, "v")]


def _build_program(na, nbm):
    nc = bacc.Bacc(name=f"attn_w_{na}_{nbm}")

    widths = _widths(na, nbm)
    nu = len(widths)
    rows = sum(widths)
    totqs = sum(w // P for w in widths)

    qt_d = nc.dram_tensor("qt", [P, rows], F16, kind="ExternalInput")
    kt_d = nc.dram_tensor("kt", [P, nu, K], F16, kind="ExternalInput")
    vb_d = nc.dram_tensor("vb", [P, nu, NKT, D + 1], F16, kind="ExternalInput")
    out_d = nc.dram_tensor("out", [totqs, P, D], F16, kind="ExternalOutput")

    with tile.TileContext(nc) as tc, ExitStack() as ctx:
        sing = ctx.enter_context(tc.tile_pool(name="sing", bufs=1))
        e1pool = ctx.enter_context(tc.tile_pool(name="e1pool", bufs=3))
        e2pool = ctx.enter_context(tc.tile_pool(name="e2pool", bufs=3))
        opool = ctx.enter_context(tc.tile_pool(name="opool", bufs=4))
        rpool = ctx.enter_context(tc.tile_pool(name="rpool", bufs=4))
        ps_s = ctx.enter_context(tc.tile_pool(name="ps_s", bufs=2, space="PSUM"))
        ps_pv = ctx.enter_context(tc.tile_pool(name="ps_pv", bufs=2, space="PSUM"))

        kt_sb = sing.tile([P, nu, K], F16)
        vb_sb = sing.tile([P, nu, NKT, D + 1], F16)
        qt_sb = sing.tile([P, rows], F16)

        # PE warmup: dependency-free matmuls on zeroed scratch, run during
        # the DMA head so the p-state ramp is paid on garbage work.
        ws = sing.tile([P, P], F16)
        nc.gpsimd.memset(ws[:, :], 0)
        ps_w = ps_s.tile([P, 3, WBIG], F32, tag="ps", name="ps_warm")
        for _ in range(NWARM):
            nc.tensor.matmul(ps_w[:, 0, 0:P], lhsT=ws[:, :], rhs=ws[:, :])

        # input DMAs: unit-0 critical pieces first (first ktile of kT and
        # its qT), then the rest; vb on the gpsimd queue.
        w0 = widths[0]
        nc.sync.dma_start(out=kt_sb[:, 0, 0:P], in_=kt_d[:, 0, 0:P])
        nc.sync.dma_start(out=qt_sb[:, 0:w0], in_=qt_d[:, 0:w0])
        nc.sync.dma_start(out=kt_sb[:, 0, P:K], in_=kt_d[:, 0, P:K])
        nc.gpsimd.dma_start(out=vb_sb[:, 0, :, :], in_=vb_d[:, 0, :, :])
        col0 = w0
        for u, w in list(enumerate(widths))[1:]:
            nc.sync.dma_start(out=kt_sb[:, u, :], in_=kt_d[:, u, :])
            nc.sync.dma_start(out=qt_sb[:, col0:col0 + w],
                              in_=qt_d[:, col0:col0 + w])
            nc.gpsimd.dma_start(out=vb_sb[:, u, :, :], in_=vb_d[:, u, :, :])
            col0 += w

        # pending PV drains: {e1, e2, u, nqs, oqs, j}
        pending = []

        def pv_lhs(rec, kt, j):
            if kt < SPLIT:
                return rec["e1"][:, kt, j * P:(j + 1) * P]
            return rec["e2"][:, kt - SPLIT, j * P:(j + 1) * P].bitcast(F16)

        def emit_pv_step():
            if not pending:
                return
            rec = pending[0]
            j = rec["j"]
            u = rec["u"]
            pv = ps_pv.tile([P, D + 1], F32, tag="pv")
            for kt in range(NKT):
                nc.tensor.matmul(
                    pv,
                    lhsT=pv_lhs(rec, kt, j),
                    rhs=vb_sb[:, u, kt, :],
                    start=(kt == 0),
                    stop=(kt == NKT - 1),
                )
            recip = rpool.tile([P, 1], F32, tag="recip")
            nc.vector.reciprocal(recip, pv[:, D:D + 1])
            o_sb = opool.tile([P, D], F16, tag="o")
            nc.vector.tensor_scalar_mul(o_sb, in0=pv[:, 0:D], scalar1=recip)
            nc.gpsimd.dma_start(out=out_d[rec["oqs"] + j, :, :], in_=o_sb)
            rec["j"] += 1
            if rec["j"] == rec["nqs"]:
                pending.pop(0)

        col = 0
        oqs = 0
        last_pv = None
        for u, w in enumerate(widths):
            nqs = w // P
            last = (u == nu - 1) and nqs == 1
            e1 = e1pool.tile([P, SPLIT, w], F16, tag="e1", name=f"e1_{u}")
            e2 = e2pool.tile([P, NKT - SPLIT, w], I16, tag="e2", name=f"e2_{u}")
            kt0 = 0
            for chn, eng in _chunks_for(w):
                ps = ps_s.tile([P, chn, w], F32, tag="ps", name=f"ps{u}_{kt0}")
                for j in range(chn):
                    kt = kt0 + j
                    nc.tensor.matmul(
                        ps[:, j, :],
                        lhsT=kt_sb[:, u, kt * P:(kt + 1) * P],
                        rhs=qt_sb[:, col:col + w],
                    )
                if eng == "s":
                    nc.scalar.activation(
                        out=e1[:, kt0:kt0 + chn, :],
                        in_=ps,
                        func=mybir.ActivationFunctionType.Exp,
                        scale=SCALE,
                    )
                else:
                    nc.vector.tensor_scalar(
                        out=e2[:, kt0 - SPLIT:kt0 - SPLIT + chn, :],
                        in0=ps,
                        scalar1=EXPA,
                        scalar2=EXPB,
                        op0=mybir.AluOpType.mult,
                        op1=mybir.AluOpType.add,
                    )
                kt0 += chn
                emit_pv_step()
                if last:
                    # drain the last unit's PV chunk-by-chunk as each act
                    # lands so the tail after the final chunk is short
                    if last_pv is None:
                        last_pv = ps_pv.tile([P, D + 1], F32, tag="pv")
                    for kt in range(kt0 - chn, kt0):
                        lhsT = (e1[:, kt, 0:P] if kt < SPLIT
                                else e2[:, kt - SPLIT, 0:P].bitcast(F16))
                        nc.tensor.matmul(
                            last_pv,
                            lhsT=lhsT,
                            rhs=vb_sb[:, u, kt, :],
                            start=(kt == 0),
                            stop=(kt == NKT - 1),
                        )
            if last:
                recip = rpool.tile([P, 1], F32, tag="recip")
                nc.vector.reciprocal(recip, last_pv[:, D:D + 1])
                o_sb = opool.tile([P, D], F16, tag="o")
                nc.vector.tensor_scalar_mul(
                    o_sb, in0=last_pv[:, 0:D], scalar1=recip)
                nc.sync.dma_start(out=out_d[oqs, :, :], in_=o_sb)
            else:
                pending.append(
                    {"e1": e1, "e2": e2, "u": u, "nqs": nqs, "oqs": oqs,
                     "j": 0})
            oqs += nqs
            col += w
        while pending:
            emit_pv_step()
    nc.compile()
    return nc


_NC_CACHE = {}


def _get_nc(na, nbm):
    key = (na, nbm)
    if key not in _NC_CACHE:
        _NC_CACHE[key] = _build_program(*key)
    return _NC_CACHE[key]


# ----------------------------------------------------------------------------
# top-level kernel
# ----------------------------------------------------------------------------

def _run(inputs: dict, trace: bool = False):
    q32 = np.ascontiguousarray(np.asarray(inputs["queries"], dtype=np.float32))
    k32 = np.ascontiguousarray(np.asarray(inputs["keys"], dtype=np.float32))
    v32 = np.ascontiguousarray(np.asarray(inputs["values"], dtype=np.float32))
    vl = np.asarray(inputs["valid_lens"]).astype(np.int64)

    if int(vl.sum()) == 0:           # every row masked: output is mean(V)
        meanv = v32.mean(axis=1)
        return np.broadcast_to(meanv[:, None, :], (B, Q, D)).copy(), None

    na, nbm, cores = _plan(vl)
    nc = _get_nc(na, nbm)

    keysT16 = np.ascontiguousarray(
        k32.transpose(0, 2, 1).astype(np.float16))          # [B,128,K]
    vb16 = np.ones((B, P, NKT, D + 1), dtype=np.float16)
    vb16[:, :, :, :D] = (
        v32.reshape(B, NKT, P, D).transpose(0, 2, 1, 3).astype(np.float16))

    in_maps = [
        _core_arrays(units, keysT16, vb16, q32, vl) for units in cores
    ]
    res = run_bass_kernel_spmd(
        nc, in_maps, core_ids=list(range(NCORES)), trace=trace)

    meanv = v32.mean(axis=1)                                 # [B, D]
    out = np.broadcast_to(meanv[:, None, :], (B, Q, D)).copy()
    for c, units in enumerate(cores):
        dev = res.results[c]["out"].astype(np.float32)       # [totqs,128,128]
        qs = 0
        for b, r0, w in units:
            for j in range(w // P):
                if b is not None:
                    lo = r0 + j * P
                    hi = min(int(vl[b]), lo + P)
                    if hi > lo:
                        out[b, lo:hi, :] = dev[qs, 0:hi - lo, :]
                qs += 1
    return out, res


def kernel(**inputs) -> np.ndarray:
    out, _ = _run(inputs, trace=False)
    return out


# revision 5
# speedup vs baseline: 1.2782x; 1.2782x over previous
"""Masked dot-product attention on 8 Trainium2 NeuronCores (valid-rows-only).

Full inputs: queries/keys/values [16, 2048, 128] f32, valid_lens [16] int.
The reference masks whole query rows q >= valid_len (softmax of a constant
row = uniform weights = mean(V)), so only sum(valid_lens) ~ 47% of rows
need real attention.  Masked rows are filled with mean(V) on the host.

Device-side plan (single SPMD program shared by all 8 cores):
  - Host cuts each batch's valid rows into 512-row and 128-row sections
    and deals them to cores so every core runs the same static unit list
    (one 128-unit first for a fast first matmul, then the 512-units, then
    the remaining 128-units; a few padded units).
  - Per-core data is pre-staged by the host in fp16: qT [128d, ROWS],
    and per-unit copies of that unit's K^T [128d, 2048k] and
    [V|1] [128k, 16kt, 129].
  - Per unit: S^T[k,q] = kT . qT on PE into psum chunks; exp of ktiles
    0-11 on ScalarE (exact spline), ktiles 12-15 on DVE via a one-
    instruction Schraudolph approximation (int16(s*A+B) bitcast as fp16,
    mean-centered so the softmax normalization cancels the bias); PE
    computes PV[q,129] = E_chunk^T . [V|1]; DVE divides by the last
    column; fp16 out, descrambled on the host.
  - A dozen dependency-free warmup matmuls on scratch SBUF run during
    the DMA head so the PE p-state is ramped before real work arrives.
"""

import math
from contextlib import ExitStack

import numpy as np

import concourse.bacc as bacc
import concourse.tile as tile
from concourse import mybir
from concourse.bass_utils import run_bass_kernel_spmd

B, Q, K, D = 16, 2048, 2048, 128
NCORES = 8
P = 128
NKT = K // P                 # 16 k-tiles
WBIG = 512                   # big unit width (4 qsubs)
WSM = 128                    # small unit width
SCALE = 1.0 / math.sqrt(D)
SPLIT = 12                   # ktiles 0..SPLIT-1 on ScalarE, rest on DVE

# fp16 Schraudolph: bits = round(s*EXPA + EXPB); bitcast(bits) ~ exp(s*SCALE)
# EXPB is lowered by the mean log2 interpolation error (2 - 1/ln2 - 1/2) so
# the sawtooth error is zero-mean and cancels against the exact-exp ktiles
# in the softmax normalization.
EXPA = 1024.0 / math.log(2.0) * SCALE
EXPB = 1024.0 * (15.0 - (2.0 - 1.0 / math.log(2.0) - 0.5))

F32 = mybir.dt.float32
F16 = mybir.dt.float16
I16 = mybir.dt.int16

NWARM = 0                    # warmup matmuls (128 cols each); 0 = disabled
SMALL_FIRST = False          # put one 128-unit first for a faster first matmul


# ----------------------------------------------------------------------------
# host-side planning
# ----------------------------------------------------------------------------

def _sections(vl):
    out = []
    for v in vl:
        a = v // WBIG
        rem = v - a * WBIG
        nb_ = (rem + WSM - 1) // WSM
        if nb_ * WSM >= WBIG:
            a += 1
            nb_ = 0
        out.append((a, nb_))
    return out


def _plan(valid_lens):
    """Returns (na, nbm, cores); cores[c] = unit list (batch, row0, width),
    batch None for padding, ordered [small, big*na, small*(nbm-1)]."""
    vl = [int(v) for v in valid_lens]
    ab = _sections(vl)
    A = sum(a for a, _ in ab)
    Bs = sum(b for _, b in ab)
    best = None
    for na in range((A + NCORES - 1) // NCORES, -1, -1):
        d = max(0, A - NCORES * na)
        btot = Bs + 4 * d
        nbm = (btot + NCORES - 1) // NCORES
        cost = NCORES * (WBIG * na + WSM * nbm)
        if cost >= WBIG * A + WSM * Bs:
            if best is None or cost < best[0]:
                best = (cost, na, nbm, d)
    _, na, nbm, d = best
    ab = list(ab)
    for b in sorted(range(B), key=lambda x: -ab[x][0]):
        if d == 0:
            break
        a, s = ab[b]
        if a > 0:
            ab[b] = (a - 1, s + 4)
            d -= 1
    big_secs, sm_secs = [], []
    for b in range(B):
        a, s = ab[b]
        for i in range(a):
            big_secs.append((b, i * WBIG))
        for i in range(s):
            sm_secs.append((b, a * WBIG + i * WSM))
    cores = []
    bi = si = 0
    for c in range(NCORES):
        bigs, smalls = [], []
        for _ in range(na):
            bigs.append((*big_secs[bi], WBIG) if bi < len(big_secs)
                        else (None, 0, WBIG))
            bi += 1
        for _ in range(nbm):
            smalls.append((*sm_secs[si], WSM) if si < len(sm_secs)
                          else (None, 0, WSM))
            si += 1
        # optionally one small unit first (fast first matmul), then bigs,
        # then the rest; default big-first (matches v1's proven DMA overlap)
        if smalls and SMALL_FIRST:
            cores.append([smalls[0]] + bigs + smalls[1:])
        else:
            cores.append(bigs + smalls)
    return na, nbm, cores


def _widths(na, nbm):
    if nbm > 0 and SMALL_FIRST:
        return [WSM] + [WBIG] * na + [WSM] * (nbm - 1)
    return [WBIG] * na + [WSM] * nbm


def _core_arrays(units, keysT16, vb16, q32, vl):
    rows = sum(w for _, _, w in units)
    nu = len(units)
    qT = np.zeros((P, rows), dtype=np.float16)
    kT = np.zeros((P, nu, K), dtype=np.float16)
    vb = np.zeros((P, nu, NKT, D + 1), dtype=np.float16)
    col = 0
    for i, (b, r0, w) in enumerate(units):
        if b is not None:
            kT[:, i, :] = keysT16[b]
            vb[:, i, :, :] = vb16[b]
            nr = max(0, min(w, vl[b] - r0))
            if nr > 0:
                qT[:, col:col + nr] = q32[b, r0:r0 + nr, :].T
        else:
            vb[:, i, :, D] = 1.0      # keep denominators nonzero on padding
        col += w
    return {"qt": qT, "kt": kT, "vb": vb}


# ----------------------------------------------------------------------------
# bass program (shared across cores; depends only on (na, nbm))
# ----------------------------------------------------------------------------

def _chunks_for(w):
    """(n_ktiles, engine) chunk list; 's' = ScalarE exact exp,
    'v' = DVE Schraudolph.  Scalar ktiles must total SPLIT."""
    if w == WBIG:
        return [(3, "s"), (3, "s"), (3, "s"), (3, "s"), (2, "v"), (2, "v")]
    return [(8, "s"), (4, "s"), (4, "v")]


def _build_program(na, nbm):
    nc = bacc.Bacc(name=f"attn_w_{na}_{nbm}")

    widths = _widths(na, nbm)
    nu = len(widths)
    rows = sum(widths)
    totqs = sum(w // P for w in widths)

    qt_d = nc.dram_tensor("qt", [P, rows], F16, kind="ExternalInput")
    kt_d = nc.dram_tensor("kt", [P, nu, K], F16, kind="ExternalInput")
    vb_d = nc.dram_tensor("vb", [P, nu, NKT, D + 1], F16, kind="ExternalInput")
    out_d = nc.dram_tensor("out", [totqs, P, D], F16, kind="ExternalOutput")

    with tile.TileContext(nc) as tc, ExitStack() as ctx:
        sing = ctx.enter_context(tc.tile_pool(name="sing", bufs=1))
        e1pool = ctx.enter_context(tc.tile_pool(name="e1pool", bufs=3))
        e2pool = ctx.enter_context(tc.tile_pool(name="e2pool", bufs=3))
        opool = ctx.enter_context(tc.tile_pool(name="opool", bufs=4))
        rpool = ctx.enter_context(tc.tile_pool(name="rpool", bufs=4))
        ps_s = ctx.enter_context(tc.tile_pool(name="ps_s", bufs=2, space="PSUM"))
        ps_pv = ctx.enter_context(tc.tile_pool(name="ps_pv", bufs=2, space="PSUM"))

        kt_sb = sing.tile([P, nu, K], F16)
        vb_sb = sing.tile([P, nu, NKT, D + 1], F16)
        qt_sb = sing.tile([P, rows], F16)

        # PE warmup: dependency-free matmuls on zeroed scratch, run during
        # the DMA head so the HAM ramp is paid on garbage work.
        if NWARM:
            ws = sing.tile([P, P], F16)
            nc.gpsimd.memset(ws[:, :], 0)
            ps_w = ps_s.tile([P, 3, WBIG], F32, tag="ps", name="ps_warm")
            for _ in range(NWARM):
                nc.tensor.matmul(ps_w[:, 0, 0:P], lhsT=ws[:, :], rhs=ws[:, :])

        # unit 0 head: first kT chunk piece + its qT land first so the first
        # S matmuls start as early as possible; V|1 is only needed at PV time.
        ch0 = _chunks_for(widths[0])[0][0]
        nc.sync.dma_start(out=kt_sb[:, 0, 0:ch0 * P], in_=kt_d[:, 0, 0:ch0 * P])
        nc.sync.dma_start(out=qt_sb[:, 0:widths[0]], in_=qt_d[:, 0:widths[0]])
        nc.sync.dma_start(out=kt_sb[:, 0, ch0 * P:K], in_=kt_d[:, 0, ch0 * P:K])
        nc.sync.dma_start(out=vb_sb[:, 0, :, :], in_=vb_d[:, 0, :, :])
        col0 = widths[0]
        for u, w in list(enumerate(widths))[1:]:
            nc.sync.dma_start(out=kt_sb[:, u, :], in_=kt_d[:, u, :])
            nc.sync.dma_start(out=vb_sb[:, u, :, :], in_=vb_d[:, u, :, :])
            nc.sync.dma_start(out=qt_sb[:, col0:col0 + w],
                              in_=qt_d[:, col0:col0 + w])
            col0 += w

        # pending PV drains: {e1, e2, u, nqs, oqs, j}
        pending = []

        def pv_lhs(rec, kt, j):
            if kt < SPLIT:
                return rec["e1"][:, kt, j * P:(j + 1) * P]
            return rec["e2"][:, kt - SPLIT, j * P:(j + 1) * P].bitcast(F16)

        def emit_pv_step():
            if not pending:
                return
            rec = pending[0]
            j = rec["j"]
            u = rec["u"]
            pv = ps_pv.tile([P, D + 1], F32, tag="pv")
            for kt in range(NKT):
                nc.tensor.matmul(
                    pv,
                    lhsT=pv_lhs(rec, kt, j),
                    rhs=vb_sb[:, u, kt, :],
                    start=(kt == 0),
                    stop=(kt == NKT - 1),
                )
            recip = rpool.tile([P, 1], F32, tag="recip")
            nc.vector.reciprocal(recip, pv[:, D:D + 1])
            o_sb = opool.tile([P, D], F16, tag="o")
            nc.vector.tensor_scalar_mul(o_sb, in0=pv[:, 0:D], scalar1=recip)
            nc.gpsimd.dma_start(out=out_d[rec["oqs"] + j, :, :], in_=o_sb)
            rec["j"] += 1
            if rec["j"] == rec["nqs"]:
                pending.pop(0)

        col = 0
        oqs = 0
        last_pv = None
        for u, w in enumerate(widths):
            nqs = w // P
            last = (u == nu - 1) and nqs == 1
            e1 = e1pool.tile([P, SPLIT, w], F16, tag="e1", name=f"e1_{u}")
            e2 = e2pool.tile([P, NKT - SPLIT, w], I16, tag="e2", name=f"e2_{u}")
            kt0 = 0
            for chn, eng in _chunks_for(w):
                ps = ps_s.tile([P, chn, w], F32, tag="ps", name=f"ps{u}_{kt0}")
                for j in range(chn):
                    kt = kt0 + j
                    nc.tensor.matmul(
                        ps[:, j, :],
                        lhsT=kt_sb[:, u, kt * P:(kt + 1) * P],
                        rhs=qt_sb[:, col:col + w],
                    )
                if eng == "s":
                    nc.scalar.activation(
                        out=e1[:, kt0:kt0 + chn, :],
                        in_=ps,
                        func=mybir.ActivationFunctionType.Exp,
                        scale=SCALE,
                    )
                else:
                    nc.vector.tensor_scalar(
                        out=e2[:, kt0 - SPLIT:kt0 - SPLIT + chn, :],
                        in0=ps,
                        scalar1=EXPA,
                        scalar2=EXPB,
                        op0=mybir.AluOpType.mult,
                        op1=mybir.AluOpType.add,
                    )
                kt0 += chn
                emit_pv_step()
                if last:
                    # drain the last unit's PV chunk-by-chunk as each act
                    # lands so the tail after the final chunk is short
                    if last_pv is None:
                        last_pv = ps_pv.tile([P, D + 1], F32, tag="pv")
                    for kt in range(kt0 - chn, kt0):
                        lhsT = (e1[:, kt, 0:P] if kt < SPLIT
                                else e2[:, kt - SPLIT, 0:P].bitcast(F16))
                        nc.tensor.matmul(
                            last_pv,
                            lhsT=lhsT,
                            rhs=vb_sb[:, u, kt, :],
                            start=(kt == 0),
                            stop=(kt == NKT - 1),
                        )
            if last:
                recip = rpool.tile([P, 1], F32, tag="recip")
                nc.vector.reciprocal(recip, last_pv[:, D:D + 1])
                o_sb = opool.tile([P, D], F16, tag="o")
                nc.vector.tensor_scalar_mul(
                    o_sb, in0=last_pv[:, 0:D], scalar1=recip)
                nc.sync.dma_start(out=out_d[oqs, :, :], in_=o_sb)
            else:
                pending.append(
                    {"e1": e1, "e2": e2, "u": u, "nqs": nqs, "oqs": oqs,
                     "j": 0})
            oqs += nqs
            col += w
        while pending:
            emit_pv_step()
    nc.compile()
    return nc


_NC_CACHE = {}


def _get_nc(na, nbm):
    key = (na, nbm)
    if key not in _NC_CACHE:
        _NC_CACHE[key] = _build_program(*key)
    return _NC_CACHE[key]


# ----------------------------------------------------------------------------
# top-level kernel
# ----------------------------------------------------------------------------

def _run(inputs: dict, trace: bool = False):
    q32 = np.ascontiguousarray(np.asarray(inputs["queries"], dtype=np.float32))
    k32 = np.ascontiguousarray(np.asarray(inputs["keys"], dtype=np.float32))
    v32 = np.ascontiguousarray(np.asarray(inputs["values"], dtype=np.float32))
    vl = np.asarray(inputs["valid_lens"]).astype(np.int64)

    if int(vl.sum()) == 0:           # every row masked: output is mean(V)
        meanv = v32.mean(axis=1)
        return np.broadcast_to(meanv[:, None, :], (B, Q, D)).copy(), None

    na, nbm, cores = _plan(vl)
    nc = _get_nc(na, nbm)

    keysT16 = np.ascontiguousarray(
        k32.transpose(0, 2, 1).astype(np.float16))          # [B,128,K]
    vb16 = np.ones((B, P, NKT, D + 1), dtype=np.float16)
    vb16[:, :, :, :D] = (
        v32.reshape(B, NKT, P, D).transpose(0, 2, 1, 3).astype(np.float16))

    in_maps = [
        _core_arrays(units, keysT16, vb16, q32, vl) for units in cores
    ]
    res = run_bass_kernel_spmd(
        nc, in_maps, core_ids=list(range(NCORES)), trace=trace)

    meanv = v32.mean(axis=1)                                 # [B, D]
    out = np.broadcast_to(meanv[:, None, :], (B, Q, D)).copy()
    for c, units in enumerate(cores):
        dev = res.results[c]["out"].astype(np.float32)       # [totqs,128,128]
        qs = 0
        for b, r0, w in units:
            for j in range(w // P):
                if b is not None:
                    lo = r0 + j * P
                    hi = min(int(vl[b]), lo + P)
                    if hi > lo:
                        out[b, lo:hi, :] = dev[qs, 0:hi - lo, :]
                qs += 1
    return out, res


def kernel(**inputs) -> np.ndarray:
    out, _ = _run(inputs, trace=False)
    return out


# revision 9
# speedup vs baseline: 1.2992x; 1.0165x over previous
"""Masked dot-product attention on 8 Trainium2 NeuronCores (valid-rows-only).

Full inputs: queries/keys/values [16, 2048, 128] f32, valid_lens [16] int.
The reference masks whole query rows q >= valid_len (softmax of a constant
row = uniform weights = mean(V)), so only sum(valid_lens) ~ 47% of rows
need real attention.  Masked rows are filled with mean(V) on the host.

Device-side plan (single SPMD program shared by all 8 cores):
  - Host cuts each batch's valid rows into 512-row and 128-row sections
    and deals them to cores so every core runs the same static unit list
    (one 128-unit first for a fast first matmul, then the 512-units, then
    the remaining 128-units; a few padded units).
  - Per-core data is pre-staged by the host in fp16: qT [128d, ROWS],
    and per-unit copies of that unit's K^T [128d, 2048k] and
    [V|1] [128k, 16kt, 129].
  - Per unit: S^T[k,q] = kT . qT on PE into psum chunks; exp of ktiles
    0-11 on ScalarE (exact spline), ktiles 12-15 on DVE via a one-
    instruction Schraudolph approximation (int16(s*A+B) bitcast as fp16,
    mean-centered so the softmax normalization cancels the bias); PE
    computes PV[q,129] = E_chunk^T . [V|1]; DVE divides by the last
    column; fp16 out, descrambled on the host.
  - A dozen dependency-free warmup matmuls on scratch SBUF run during
    the DMA head so the PE p-state is ramped before real work arrives.
"""

import math
from contextlib import ExitStack

import numpy as np

import concourse.bacc as bacc
import concourse.tile as tile
from concourse import mybir
from concourse.bass_utils import run_bass_kernel_spmd

B, Q, K, D = 16, 2048, 2048, 128
NCORES = 8
P = 128
NKT = K // P                 # 16 k-tiles
WBIG = 512                   # big unit width (4 qsubs)
WSM = 128                    # small unit width
SCALE = 1.0 / math.sqrt(D)
SPLIT = 12                   # ktiles 0..SPLIT-1 on ScalarE, rest on DVE

# fp16 Schraudolph: bits = round(s*EXPA + EXPB); bitcast(bits) ~ exp(s*SCALE)
# EXPB is lowered by the mean log2 interpolation error (2 - 1/ln2 - 1/2) so
# the sawtooth error is zero-mean and cancels against the exact-exp ktiles
# in the softmax normalization.
EXPA = 1024.0 / math.log(2.0) * SCALE
EXPB = 1024.0 * (15.0 - (2.0 - 1.0 / math.log(2.0) - 0.5))

F32 = mybir.dt.float32
F16 = mybir.dt.float16
I16 = mybir.dt.int16

NWARM = 0                    # warmup matmuls (128 cols each); 0 = disabled
SMALL_FIRST = False          # put one 128-unit first for a faster first matmul


# ----------------------------------------------------------------------------
# host-side planning
# ----------------------------------------------------------------------------

def _sections(vl):
    out = []
    for v in vl:
        a = v // WBIG
        rem = v - a * WBIG
        nb_ = (rem + WSM - 1) // WSM
        if nb_ * WSM >= WBIG:
            a += 1
            nb_ = 0
        out.append((a, nb_))
    return out


def _plan(valid_lens):
    """Returns (na, nbm, cores); cores[c] = unit list (batch, row0, width),
    batch None for padding, ordered [small, big*na, small*(nbm-1)]."""
    vl = [int(v) for v in valid_lens]
    ab = _sections(vl)
    A = sum(a for a, _ in ab)
    Bs = sum(b for _, b in ab)
    best = None
    for na in range((A + NCORES - 1) // NCORES, -1, -1):
        d = max(0, A - NCORES * na)
        btot = Bs + 4 * d
        nbm = (btot + NCORES - 1) // NCORES
        cost = NCORES * (WBIG * na + WSM * nbm)
        if cost >= WBIG * A + WSM * Bs:
            if best is None or cost < best[0]:
                best = (cost, na, nbm, d)
    _, na, nbm, d = best
    ab = list(ab)
    for b in sorted(range(B), key=lambda x: -ab[x][0]):
        if d == 0:
            break
        a, s = ab[b]
        if a > 0:
            ab[b] = (a - 1, s + 4)
            d -= 1
    big_secs, sm_secs = [], []
    for b in range(B):
        a, s = ab[b]
        for i in range(a):
            big_secs.append((b, i * WBIG))
        for i in range(s):
            sm_secs.append((b, a * WBIG + i * WSM))
    cores = []
    bi = si = 0
    for c in range(NCORES):
        bigs, smalls = [], []
        for _ in range(na):
            bigs.append((*big_secs[bi], WBIG) if bi < len(big_secs)
                        else (None, 0, WBIG))
            bi += 1
        for _ in range(nbm):
            smalls.append((*sm_secs[si], WSM) if si < len(sm_secs)
                          else (None, 0, WSM))
            si += 1
        # optionally one small unit first (fast first matmul), then bigs,
        # then the rest; default big-first (matches v1's proven DMA overlap)
        if smalls and SMALL_FIRST:
            cores.append([smalls[0]] + bigs + smalls[1:])
        else:
            cores.append(bigs + smalls)
    return na, nbm, cores


def _widths(na, nbm):
    if nbm > 0 and SMALL_FIRST:
        return [WSM] + [WBIG] * na + [WSM] * (nbm - 1)
    return [WBIG] * na + [WSM] * nbm


def _core_arrays(units, keysT16, vb16, q32, vl):
    rows = sum(w for _, _, w in units)
    nu = len(units)
    qT = np.zeros((P, rows), dtype=np.float16)
    kT = np.zeros((P, nu, K), dtype=np.float16)
    vb = np.zeros((P, nu, NKT, D + 1), dtype=np.float16)
    col = 0
    for i, (b, r0, w) in enumerate(units):
        if b is not None:
            kT[:, i, :] = keysT16[b]
            vb[:, i, :, :] = vb16[b]
            nr = max(0, min(w, vl[b] - r0))
            if nr > 0:
                qT[:, col:col + nr] = q32[b, r0:r0 + nr, :].T
        else:
            vb[:, i, :, D] = 1.0      # keep denominators nonzero on padding
        col += w
    return {"qt": qT, "kt": kT, "vb": vb}


# ----------------------------------------------------------------------------
# bass program (shared across cores; depends only on (na, nbm))
# ----------------------------------------------------------------------------

def _chunks_for(w, last=False):
    """(n_ktiles, engine) chunk list; 's' = ScalarE exact exp,
    'v' = DVE Schraudolph.  2-ktile chunks (2 PSUM banks) so ps_s can
    triple-buffer; ktiles < SPLIT land in e1, >= SPLIT in e2 regardless
    of producer.  The last unit is scalar-only so DVE is free to run the
    final normalize immediately."""
    if w == WBIG:
        return [(2, "s")] * 6 + [(2, "v")] * 2
    if last:
        return [(4, "s")] * 4
    return [(4, "s")] * 3 + [(4, "v")]


def _build_program(na, nbm):
    nc = bacc.Bacc(name=f"attn_w_{na}_{nbm}")

    widths = _widths(na, nbm)
    nu = len(widths)
    rows = sum(widths)
    totqs = sum(w // P for w in widths)

    qt_d = nc.dram_tensor("qt", [P, rows], F16, kind="ExternalInput")
    kt_d = nc.dram_tensor("kt", [P, nu, K], F16, kind="ExternalInput")
    vb_d = nc.dram_tensor("vb", [P, nu, NKT, D + 1], F16, kind="ExternalInput")
    out_d = nc.dram_tensor("out", [totqs, P, D], F16, kind="ExternalOutput")

    with tile.TileContext(nc) as tc, ExitStack() as ctx:
        sing = ctx.enter_context(tc.tile_pool(name="sing", bufs=1))
        e1pool = ctx.enter_context(tc.tile_pool(name="e1pool", bufs=3))
        e2pool = ctx.enter_context(tc.tile_pool(name="e2pool", bufs=3))
        opool = ctx.enter_context(tc.tile_pool(name="opool", bufs=4))
        rpool = ctx.enter_context(tc.tile_pool(name="rpool", bufs=4))
        ps_s = ctx.enter_context(tc.tile_pool(name="ps_s", bufs=3, space="PSUM"))
        ps_pv = ctx.enter_context(tc.tile_pool(name="ps_pv", bufs=2, space="PSUM"))

        kt_sb = sing.tile([P, nu, K], F16)
        vb_sb = sing.tile([P, nu, NKT, D + 1], F16)
        qt_sb = sing.tile([P, rows], F16)

        # PE warmup: dependency-free matmuls on zeroed scratch, run during
        # the DMA head so the HAM ramp is paid on garbage work.
        if NWARM:
            ws = sing.tile([P, P], F16)
            nc.gpsimd.memset(ws[:, :], 0)
            ps_w = ps_s.tile([P, 2, WBIG], F32, tag="ps", name="ps_warm")
            for _ in range(NWARM):
                nc.tensor.matmul(ps_w[:, 0, 0:P], lhsT=ws[:, :], rhs=ws[:, :])

        # unit 0 head: land the first k-tile and the first 128 qT columns
        # first (the opening 128-col strip matmul needs only those), then
        # progressively bigger pieces; V|1 is only needed at PV time.
        w0 = widths[0]
        nc.sync.dma_start(out=kt_sb[:, 0, 0:P], in_=kt_d[:, 0, 0:P])
        nc.sync.dma_start(out=qt_sb[:, 0:P], in_=qt_d[:, 0:P])
        if w0 > P:
            nc.sync.dma_start(out=qt_sb[:, P:w0], in_=qt_d[:, P:w0])
        nc.sync.dma_start(out=kt_sb[:, 0, P:2 * P], in_=kt_d[:, 0, P:2 * P])
        nc.sync.dma_start(out=kt_sb[:, 0, 2 * P:K], in_=kt_d[:, 0, 2 * P:K])
        nc.sync.dma_start(out=vb_sb[:, 0, :, :], in_=vb_d[:, 0, :, :])
        col0 = w0
        for u, w in list(enumerate(widths))[1:]:
            nc.sync.dma_start(out=kt_sb[:, u, :], in_=kt_d[:, u, :])
            nc.sync.dma_start(out=vb_sb[:, u, :, :], in_=vb_d[:, u, :, :])
            nc.sync.dma_start(out=qt_sb[:, col0:col0 + w],
                              in_=qt_d[:, col0:col0 + w])
            col0 += w

        # pending PV drains: {e1, e2, u, nqs, oqs, j}
        pending = []

        def pv_lhs(rec, kt, j):
            if kt < SPLIT:
                return rec["e1"][:, kt, j * P:(j + 1) * P]
            return rec["e2"][:, kt - SPLIT, j * P:(j + 1) * P].bitcast(F16)

        def emit_pv_step():
            if not pending:
                return
            rec = pending[0]
            j = rec["j"]
            u = rec["u"]
            pv = ps_pv.tile([P, D + 1], F32, tag="pv")
            for kt in range(NKT):
                nc.tensor.matmul(
                    pv,
                    lhsT=pv_lhs(rec, kt, j),
                    rhs=vb_sb[:, u, kt, :],
                    start=(kt == 0),
                    stop=(kt == NKT - 1),
                )
            recip = rpool.tile([P, 1], F32, tag="recip")
            nc.vector.reciprocal(recip, pv[:, D:D + 1])
            o_sb = opool.tile([P, D], F16, tag="o")
            nc.vector.tensor_scalar_mul(o_sb, in0=pv[:, 0:D], scalar1=recip)
            nc.gpsimd.dma_start(out=out_d[rec["oqs"] + j, :, :], in_=o_sb)
            rec["j"] += 1
            if rec["j"] == rec["nqs"]:
                pending.pop(0)

        col = 0
        oqs = 0
        last_pv = None
        for u, w in enumerate(widths):
            nqs = w // P
            last = (u == nu - 1) and nqs == 1
            e1 = e1pool.tile([P, SPLIT, w], F16, tag="e1", name=f"e1_{u}")
            e2 = e2pool.tile([P, NKT - SPLIT, w], I16, tag="e2", name=f"e2_{u}")
            kt0 = 0
            for ci, (chn, eng) in enumerate(_chunks_for(w, last)):
                ps = ps_s.tile([P, chn, w], F32, tag="ps", name=f"ps{u}_{kt0}")
                for j in range(chn):
                    kt = kt0 + j
                    if u == 0 and ci == 0:
                        # 128-col strips so the very first matmul only needs
                        # the first k-tile and 128 qT columns
                        for s in range(w // P):
                            nc.tensor.matmul(
                                ps[:, j, s * P:(s + 1) * P],
                                lhsT=kt_sb[:, u, kt * P:(kt + 1) * P],
                                rhs=qt_sb[:, col + s * P:col + (s + 1) * P],
                            )
                    else:
                        nc.tensor.matmul(
                            ps[:, j, :],
                            lhsT=kt_sb[:, u, kt * P:(kt + 1) * P],
                            rhs=qt_sb[:, col:col + w],
                        )
                if eng == "s":
                    if kt0 < SPLIT:
                        e_out = e1[:, kt0:kt0 + chn, :]
                    else:
                        e_out = e2[:, kt0 - SPLIT:kt0 - SPLIT + chn, :]
                        e_out = e_out.bitcast(F16)
                    nc.scalar.activation(
                        out=e_out,
                        in_=ps,
                        func=mybir.ActivationFunctionType.Exp,
                        scale=SCALE,
                    )
                else:
                    nc.vector.tensor_scalar(
                        out=e2[:, kt0 - SPLIT:kt0 - SPLIT + chn, :],
                        in0=ps,
                        scalar1=EXPA,
                        scalar2=EXPB,
                        op0=mybir.AluOpType.mult,
                        op1=mybir.AluOpType.add,
                    )
                kt0 += chn
                emit_pv_step()
                if last:
                    # drain the last unit's PV chunk-by-chunk as each act
                    # lands so the tail after the final chunk is short
                    if last_pv is None:
                        last_pv = ps_pv.tile([P, D + 1], F32, tag="pv")
                    for kt in range(kt0 - chn, kt0):
                        lhsT = (e1[:, kt, 0:P] if kt < SPLIT
                                else e2[:, kt - SPLIT, 0:P].bitcast(F16))
                        nc.tensor.matmul(
                            last_pv,
                            lhsT=lhsT,
                            rhs=vb_sb[:, u, kt, :],
                            start=(kt == 0),
                            stop=(kt == NKT - 1),
                        )
            if last:
                recip = rpool.tile([P, 1], F32, tag="recip")
                nc.vector.reciprocal(recip, last_pv[:, D:D + 1])
                o_sb = opool.tile([P, D], F16, tag="o")
                nc.vector.tensor_scalar_mul(
                    o_sb, in0=last_pv[:, 0:D], scalar1=recip)
                nc.sync.dma_start(out=out_d[oqs, :, :], in_=o_sb)
            else:
                pending.append(
                    {"e1": e1, "e2": e2, "u": u, "nqs": nqs, "oqs": oqs,
                     "j": 0})
            oqs += nqs
            col += w
        while pending:
            emit_pv_step()
    nc.compile()
    return nc


_NC_CACHE = {}


def _get_nc(na, nbm):
    key = (na, nbm)
    if key not in _NC_CACHE:
        _NC_CACHE[key] = _build_program(*key)
    return _NC_CACHE[key]


# ----------------------------------------------------------------------------
# top-level kernel
# ----------------------------------------------------------------------------

def _run(inputs: dict, trace: bool = False):
    q32 = np.ascontiguousarray(np.asarray(inputs["queries"], dtype=np.float32))
    k32 = np.ascontiguousarray(np.asarray(inputs["keys"], dtype=np.float32))
    v32 = np.ascontiguousarray(np.asarray(inputs["values"], dtype=np.float32))
    vl = np.asarray(inputs["valid_lens"]).astype(np.int64)

    if int(vl.sum()) == 0:           # every row masked: output is mean(V)
        meanv = v32.mean(axis=1)
        return np.broadcast_to(meanv[:, None, :], (B, Q, D)).copy(), None

    na, nbm, cores = _plan(vl)
    nc = _get_nc(na, nbm)

    keysT16 = np.ascontiguousarray(
        k32.transpose(0, 2, 1).astype(np.float16))          # [B,128,K]
    vb16 = np.ones((B, P, NKT, D + 1), dtype=np.float16)
    vb16[:, :, :, :D] = (
        v32.reshape(B, NKT, P, D).transpose(0, 2, 1, 3).astype(np.float16))

    in_maps = [
        _core_arrays(units, keysT16, vb16, q32, vl) for units in cores
    ]
    res = run_bass_kernel_spmd(
        nc, in_maps, core_ids=list(range(NCORES)), trace=trace)

    meanv = v32.mean(axis=1)                                 # [B, D]
    out = np.broadcast_to(meanv[:, None, :], (B, Q, D)).copy()
    for c, units in enumerate(cores):
        dev = res.results[c]["out"].astype(np.float32)       # [totqs,128,128]
        qs = 0
        for b, r0, w in units:
            for j in range(w // P):
                if b is not None:
                    lo = r0 + j * P
                    hi = min(int(vl[b]), lo + P)
                    if hi > lo:
                        out[b, lo:hi, :] = dev[qs, 0:hi - lo, :]
                qs += 1
    return out, res


def kernel(**inputs) -> np.ndarray:
    out, _ = _run(inputs, trace=False)
    return out


# revision 17
# speedup vs baseline: 1.3138x; 1.0112x over previous
"""Masked dot-product attention on 8 Trainium2 NeuronCores (valid-rows-only).

Full inputs: queries/keys/values [16, 2048, 128] f32, valid_lens [16] int.
The reference masks whole query rows q >= valid_len (softmax of a constant
row = uniform weights = mean(V)), so only sum(valid_lens) ~ 47% of rows
need real attention.  Masked rows are filled with mean(V) on the host.

Device-side plan (single SPMD program shared by all 8 cores):
  - Host cuts each batch's valid rows into 512-row and 128-row sections
    and deals them to cores so every core runs the same static unit list
    (one 128-unit first for a fast first matmul, then the 512-units, then
    the remaining 128-units; a few padded units).
  - Per-core data is pre-staged by the host in fp16: qT [128d, ROWS],
    and per-unit copies of that unit's K^T [128d, 2048k] and
    [V|1] [128k, 16kt, 129].
  - Per unit: S^T[k,q] = kT . qT on PE into psum chunks; exp of ktiles
    0-11 on ScalarE (exact spline), ktiles 12-15 on DVE via a one-
    instruction Schraudolph approximation (int16(s*A+B) bitcast as fp16,
    mean-centered so the softmax normalization cancels the bias); PE
    computes PV[q,129] = E_chunk^T . [V|1]; DVE divides by the last
    column; fp16 out, descrambled on the host.
  - A dozen dependency-free warmup matmuls on scratch SBUF run during
    the DMA head so the PE p-state is ramped before real work arrives.
"""

import math
from contextlib import ExitStack

import numpy as np

import concourse.bacc as bacc
import concourse.tile as tile
from concourse import mybir
from concourse.bass_utils import run_bass_kernel_spmd

B, Q, K, D = 16, 2048, 2048, 128
NCORES = 8
P = 128
NKT = K // P                 # 16 k-tiles
WBIG = 512                   # big unit width (4 qsubs)
WSM = 128                    # small unit width
SCALE = 1.0 / math.sqrt(D)
SPLIT = 12                   # ktiles 0..SPLIT-1 on ScalarE, rest on DVE

# fp16 Schraudolph: bits = round(s*EXPA + EXPB); bitcast(bits) ~ exp(s*SCALE)
# EXPB is lowered by the mean log2 interpolation error (2 - 1/ln2 - 1/2) so
# the sawtooth error is zero-mean and cancels against the exact-exp ktiles
# in the softmax normalization.
EXPA = 1024.0 / math.log(2.0) * SCALE
EXPB = 1024.0 * (15.0 - (2.0 - 1.0 / math.log(2.0) - 0.5))

F32 = mybir.dt.float32
F16 = mybir.dt.float16
I16 = mybir.dt.int16

NWARM = 0                    # warmup matmuls (128 cols each); 0 = disabled
SMALL_FIRST = False          # put one 128-unit first for a faster first matmul


# ----------------------------------------------------------------------------
# host-side planning
# ----------------------------------------------------------------------------

def _sections(vl):
    out = []
    for v in vl:
        a = v // WBIG
        rem = v - a * WBIG
        nb_ = (rem + WSM - 1) // WSM
        if nb_ * WSM >= WBIG:
            a += 1
            nb_ = 0
        out.append((a, nb_))
    return out


def _plan(valid_lens):
    """Returns (na, nbm, cores); cores[c] = unit list (batch, row0, width),
    batch None for padding, ordered [small, big*na, small*(nbm-1)]."""
    vl = [int(v) for v in valid_lens]
    ab = _sections(vl)
    A = sum(a for a, _ in ab)
    Bs = sum(b for _, b in ab)
    best = None
    for na in range((A + NCORES - 1) // NCORES, -1, -1):
        d = max(0, A - NCORES * na)
        btot = Bs + 4 * d
        nbm = (btot + NCORES - 1) // NCORES
        cost = NCORES * (WBIG * na + WSM * nbm)
        if cost >= WBIG * A + WSM * Bs:
            if best is None or cost < best[0]:
                best = (cost, na, nbm, d)
    _, na, nbm, d = best
    ab = list(ab)
    for b in sorted(range(B), key=lambda x: -ab[x][0]):
        if d == 0:
            break
        a, s = ab[b]
        if a > 0:
            ab[b] = (a - 1, s + 4)
            d -= 1
    big_secs, sm_secs = [], []
    for b in range(B):
        a, s = ab[b]
        for i in range(a):
            big_secs.append((b, i * WBIG))
        for i in range(s):
            sm_secs.append((b, a * WBIG + i * WSM))
    cores = []
    bi = si = 0
    for c in range(NCORES):
        bigs, smalls = [], []
        for _ in range(na):
            bigs.append((*big_secs[bi], WBIG) if bi < len(big_secs)
                        else (None, 0, WBIG))
            bi += 1
        for _ in range(nbm):
            smalls.append((*sm_secs[si], WSM) if si < len(sm_secs)
                          else (None, 0, WSM))
            si += 1
        # optionally one small unit first (fast first matmul), then bigs,
        # then the rest; default big-first (matches v1's proven DMA overlap)
        if smalls and SMALL_FIRST:
            cores.append([smalls[0]] + bigs + smalls[1:])
        else:
            cores.append(bigs + smalls)
    return na, nbm, cores


def _widths(na, nbm):
    if nbm > 0 and SMALL_FIRST:
        return [WSM] + [WBIG] * na + [WSM] * (nbm - 1)
    return [WBIG] * na + [WSM] * nbm


def _core_arrays(units, keysT16, vb16, q32, vl):
    rows = sum(w for _, _, w in units)
    nu = len(units)
    qT = np.zeros((P, rows), dtype=np.float16)
    kT = np.zeros((P, nu, K), dtype=np.float16)
    vb = np.zeros((P, nu, NKT, D + 1), dtype=np.float16)
    col = 0
    for i, (b, r0, w) in enumerate(units):
        if b is not None:
            kT[:, i, :] = keysT16[b]
            vb[:, i, :, :] = vb16[b]
            nr = max(0, min(w, vl[b] - r0))
            if nr > 0:
                qT[:, col:col + nr] = q32[b, r0:r0 + nr, :].T
        else:
            vb[:, i, :, D] = 1.0      # keep denominators nonzero on padding
        col += w
    # header = [unit0 k-tiles 0-2 | full qT] so one DMA covers the critical
    # first-chunk inputs
    hd = np.concatenate([kT[:, 0, 0:3 * P], qT], axis=1)
    return {"hd": np.ascontiguousarray(hd), "kt": kT, "vb": vb}


# ----------------------------------------------------------------------------
# bass program (shared across cores; depends only on (na, nbm))
# ----------------------------------------------------------------------------

def _chunks_for(w, last=False):
    """(n_ktiles, engine) chunk list; 's' = ScalarE exact exp,
    'v' = DVE Schraudolph.  Scalar chunks are kept large (the 352-cycle
    ACTIVATE pipeline fill is NOT hidden between instructions, so act
    count is expensive); ktiles < SPLIT land in e1, >= SPLIT in e2
    regardless of producer.  The last unit is scalar-only so DVE is free
    to run the final normalize immediately."""
    if w == WBIG:
        return [(3, "s"), (3, "s"), (3, "s"), (3, "s"), (2, "v"), (2, "v")]
    if last:
        return [(8, "s"), (4, "s"), (4, "s")]
    return [(8, "s"), (4, "s"), (4, "v")]


def _build_program(na, nbm):
    nc = bacc.Bacc(name=f"attn_w_{na}_{nbm}")

    widths = _widths(na, nbm)
    nu = len(widths)
    rows = sum(widths)
    totqs = sum(w // P for w in widths)

    # hd = [unit0 k-tiles 0-2 | full qT]: the whole critical head lands in
    # one DMA (each dma_start pays ~1us of serialized DGE startup, so the
    # first matmul is gated by DMA COUNT ahead of it, not bytes)
    HKT = 3                      # unit-0 k-tiles packed into the header
    hd_d = nc.dram_tensor("hd", [P, HKT * P + rows], F16,
                          kind="ExternalInput")
    kt_d = nc.dram_tensor("kt", [P, nu, K], F16, kind="ExternalInput")
    vb_d = nc.dram_tensor("vb", [P, nu, NKT, D + 1], F16, kind="ExternalInput")
    out_d = nc.dram_tensor("out", [totqs, P, D], F16, kind="ExternalOutput")

    with tile.TileContext(nc) as tc, ExitStack() as ctx:
        sing = ctx.enter_context(tc.tile_pool(name="sing", bufs=1))
        e1pool = ctx.enter_context(tc.tile_pool(name="e1pool", bufs=3))
        e2pool = ctx.enter_context(tc.tile_pool(name="e2pool", bufs=3))
        opool = ctx.enter_context(tc.tile_pool(name="opool", bufs=4))
        rpool = ctx.enter_context(tc.tile_pool(name="rpool", bufs=4))
        ps_s = ctx.enter_context(tc.tile_pool(name="ps_s", bufs=2, space="PSUM"))
        ps_pv = ctx.enter_context(tc.tile_pool(name="ps_pv", bufs=2, space="PSUM"))

        HKT = 3
        kt_sb = sing.tile([P, nu, K], F16)
        vb_sb = sing.tile([P, nu, NKT, D + 1], F16)
        hd_sb = sing.tile([P, HKT * P + rows], F16)

        def qt_ap(c0, c1):                   # qT view inside the header tile
            return hd_sb[:, HKT * P + c0:HKT * P + c1]

        # PE warmup: dependency-free matmuls on zeroed scratch, run during
        # the DMA head so the HAM ramp is paid on garbage work.
        if NWARM:
            ws = sing.tile([P, P], F16)
            nc.gpsimd.memset(ws[:, :], 0)
            ps_w = ps_s.tile([P, 3, WBIG], F32, tag="ps", name="ps_warm")
            for _ in range(NWARM):
                nc.tensor.matmul(ps_w[:, 0, 0:P], lhsT=ws[:, :], rhs=ws[:, :])

        # head: one DMA carries unit-0's first 3 k-tiles plus its qT, so
        # the first S chunk is gated by a single transfer; then unit-0's
        # remaining k-tiles, the rest of qT, and per-unit kt/vb.
        w0 = widths[0]
        nc.sync.dma_start(out=hd_sb[:, 0:HKT * P + w0],
                          in_=hd_d[:, 0:HKT * P + w0])
        nc.sync.dma_start(out=kt_sb[:, 0, HKT * P:K],
                          in_=kt_d[:, 0, HKT * P:K])
        if rows > w0:
            nc.sync.dma_start(out=hd_sb[:, HKT * P + w0:],
                              in_=hd_d[:, HKT * P + w0:])
        nc.sync.dma_start(out=vb_sb[:, 0, :, :], in_=vb_d[:, 0, :, :])
        for u, w in list(enumerate(widths))[1:]:
            nc.sync.dma_start(out=kt_sb[:, u, :], in_=kt_d[:, u, :])
            nc.sync.dma_start(out=vb_sb[:, u, :, :], in_=vb_d[:, u, :, :])

        # pending PV drains: {e1, e2, u, nqs, oqs, j}
        pending = []

        def pv_lhs(rec, kt, j):
            if kt < SPLIT:
                return rec["e1"][:, kt, j * P:(j + 1) * P]
            return rec["e2"][:, kt - SPLIT, j * P:(j + 1) * P].bitcast(F16)

        def emit_pv_step():
            if not pending:
                return
            rec = pending[0]
            j = rec["j"]
            u = rec["u"]
            pv = ps_pv.tile([P, D + 1], F32, tag="pv")
            for kt in range(NKT):
                nc.tensor.matmul(
                    pv,
                    lhsT=pv_lhs(rec, kt, j),
                    rhs=vb_sb[:, u, kt, :],
                    start=(kt == 0),
                    stop=(kt == NKT - 1),
                )
            recip = rpool.tile([P, 1], F32, tag="recip")
            nc.vector.reciprocal(recip, pv[:, D:D + 1])
            o_sb = opool.tile([P, D], F16, tag="o")
            nc.vector.tensor_scalar_mul(o_sb, in0=pv[:, 0:D], scalar1=recip)
            nc.gpsimd.dma_start(out=out_d[rec["oqs"] + j, :, :], in_=o_sb)
            rec["j"] += 1
            if rec["j"] == rec["nqs"]:
                pending.pop(0)

        col = 0
        oqs = 0
        last_pv = None
        for u, w in enumerate(widths):
            nqs = w // P
            last = (u == nu - 1) and nqs == 1
            e1 = e1pool.tile([P, SPLIT, w], F16, tag="e1", name=f"e1_{u}")
            e2 = e2pool.tile([P, NKT - SPLIT, w], I16, tag="e2", name=f"e2_{u}")
            kt0 = 0
            for ci, (chn, eng) in enumerate(_chunks_for(w, last)):
                ps = ps_s.tile([P, chn, w], F32, tag="ps", name=f"ps{u}_{kt0}")
                for j in range(chn):
                    kt = kt0 + j
                    if u == 0 and kt < 3:    # unit-0 head k-tiles live in hd
                        lhsT = hd_sb[:, kt * P:(kt + 1) * P]
                    else:
                        lhsT = kt_sb[:, u, kt * P:(kt + 1) * P]
                    nc.tensor.matmul(
                        ps[:, j, :],
                        lhsT=lhsT,
                        rhs=qt_ap(col, col + w),
                    )
                if eng == "s":
                    if kt0 < SPLIT:
                        e_out = e1[:, kt0:kt0 + chn, :]
                    else:
                        e_out = e2[:, kt0 - SPLIT:kt0 - SPLIT + chn, :]
                        e_out = e_out.bitcast(F16)
                    nc.scalar.activation(
                        out=e_out,
                        in_=ps,
                        func=mybir.ActivationFunctionType.Exp,
                        scale=SCALE,
                    )
                else:
                    nc.vector.tensor_scalar(
                        out=e2[:, kt0 - SPLIT:kt0 - SPLIT + chn, :],
                        in0=ps,
                        scalar1=EXPA,
                        scalar2=EXPB,
                        op0=mybir.AluOpType.mult,
                        op1=mybir.AluOpType.add,
                    )
                kt0 += chn
                emit_pv_step()
                if last:
                    # drain the last unit's PV chunk-by-chunk as each act
                    # lands so the tail after the final chunk is short
                    if last_pv is None:
                        last_pv = ps_pv.tile([P, D + 1], F32, tag="pv")
                    for kt in range(kt0 - chn, kt0):
                        lhsT = (e1[:, kt, 0:P] if kt < SPLIT
                                else e2[:, kt - SPLIT, 0:P].bitcast(F16))
                        nc.tensor.matmul(
                            last_pv,
                            lhsT=lhsT,
                            rhs=vb_sb[:, u, kt, :],
                            start=(kt == 0),
                            stop=(kt == NKT - 1),
                        )
            if last:
                recip = rpool.tile([P, 1], F32, tag="recip")
                nc.vector.reciprocal(recip, last_pv[:, D:D + 1])
                o_sb = opool.tile([P, D], F16, tag="o")
                nc.vector.tensor_scalar_mul(
                    o_sb, in0=last_pv[:, 0:D], scalar1=recip)
                nc.sync.dma_start(out=out_d[oqs, :, :], in_=o_sb)
            else:
                pending.append(
                    {"e1": e1, "e2": e2, "u": u, "nqs": nqs, "oqs": oqs,
                     "j": 0})
            oqs += nqs
            col += w
        while pending:
            emit_pv_step()
    nc.compile()
    return nc


_NC_CACHE = {}


def _get_nc(na, nbm):
    key = (na, nbm)
    if key not in _NC_CACHE:
        _NC_CACHE[key] = _build_program(*key)
    return _NC_CACHE[key]


# ----------------------------------------------------------------------------
# top-level kernel
# ----------------------------------------------------------------------------

def _run(inputs: dict, trace: bool = False):
    q32 = np.ascontiguousarray(np.asarray(inputs["queries"], dtype=np.float32))
    k32 = np.ascontiguousarray(np.asarray(inputs["keys"], dtype=np.float32))
    v32 = np.ascontiguousarray(np.asarray(inputs["values"], dtype=np.float32))
    vl = np.asarray(inputs["valid_lens"]).astype(np.int64)

    if int(vl.sum()) == 0:           # every row masked: output is mean(V)
        meanv = v32.mean(axis=1)
        return np.broadcast_to(meanv[:, None, :], (B, Q, D)).copy(), None

    na, nbm, cores = _plan(vl)
    nc = _get_nc(na, nbm)

    keysT16 = np.ascontiguousarray(
        k32.transpose(0, 2, 1).astype(np.float16))          # [B,128,K]
    vb16 = np.ones((B, P, NKT, D + 1), dtype=np.float16)
    vb16[:, :, :, :D] = (
        v32.reshape(B, NKT, P, D).transpose(0, 2, 1, 3).astype(np.float16))

    in_maps = [
        _core_arrays(units, keysT16, vb16, q32, vl) for units in cores
    ]
    res = run_bass_kernel_spmd(
        nc, in_maps, core_ids=list(range(NCORES)), trace=trace)

    meanv = v32.mean(axis=1)                                 # [B, D]
    out = np.broadcast_to(meanv[:, None, :], (B, Q, D)).copy()
    for c, units in enumerate(cores):
        dev = res.results[c]["out"].astype(np.float32)       # [totqs,128,128]
        qs = 0
        for b, r0, w in units:
            for j in range(w // P):
                if b is not None:
                    lo = r0 + j * P
                    hi = min(int(vl[b]), lo + P)
                    if hi > lo:
                        out[b, lo:hi, :] = dev[qs, 0:hi - lo, :]
                qs += 1
    return out, res


def kernel(**inputs) -> np.ndarray:
    out, _ = _run(inputs, trace=False)
    return out


# revision 18
# speedup vs baseline: 1.3195x; 1.0044x over previous
"""Masked dot-product attention on 8 Trainium2 NeuronCores (valid-rows-only).

Full inputs: queries/keys/values [16, 2048, 128] f32, valid_lens [16] int.
The reference masks whole query rows q >= valid_len (softmax of a constant
row = uniform weights = mean(V)), so only sum(valid_lens) ~ 47% of rows
need real attention.  Masked rows are filled with mean(V) on the host.

Device-side plan (single SPMD program shared by all 8 cores):
  - Host cuts each batch's valid rows into 512-row and 128-row sections
    and deals them to cores so every core runs the same static unit list
    (one 128-unit first for a fast first matmul, then the 512-units, then
    the remaining 128-units; a few padded units).
  - Per-core data is pre-staged by the host in fp16: qT [128d, ROWS],
    and per-unit copies of that unit's K^T [128d, 2048k] and
    [V|1] [128k, 16kt, 129].
  - Per unit: S^T[k,q] = kT . qT on PE into psum chunks; exp of ktiles
    0-11 on ScalarE (exact spline), ktiles 12-15 on DVE via a one-
    instruction Schraudolph approximation (int16(s*A+B) bitcast as fp16,
    mean-centered so the softmax normalization cancels the bias); PE
    computes PV[q,129] = E_chunk^T . [V|1]; DVE divides by the last
    column; fp16 out, descrambled on the host.
  - A dozen dependency-free warmup matmuls on scratch SBUF run during
    the DMA head so the PE p-state is ramped before real work arrives.
"""

import math
from contextlib import ExitStack

import numpy as np

import concourse.bacc as bacc
import concourse.tile as tile
from concourse import mybir
from concourse.bass_utils import run_bass_kernel_spmd

B, Q, K, D = 16, 2048, 2048, 128
NCORES = 8
P = 128
NKT = K // P                 # 16 k-tiles
WBIG = 512                   # big unit width (4 qsubs)
WSM = 128                    # small unit width
SCALE = 1.0 / math.sqrt(D)
SPLIT = 12                   # ktiles 0..SPLIT-1 on ScalarE, rest on DVE

# fp16 Schraudolph: bits = round(s*EXPA + EXPB); bitcast(bits) ~ exp(s*SCALE)
# EXPB is lowered by the mean log2 interpolation error (2 - 1/ln2 - 1/2) so
# the sawtooth error is zero-mean and cancels against the exact-exp ktiles
# in the softmax normalization.
EXPA = 1024.0 / math.log(2.0) * SCALE
EXPB = 1024.0 * (15.0 - (2.0 - 1.0 / math.log(2.0) - 0.5))

F32 = mybir.dt.float32
F16 = mybir.dt.float16
I16 = mybir.dt.int16

NWARM = 16                   # warmup matmuls (128 cols each); 0 = disabled
SMALL_FIRST = False          # put one 128-unit first for a faster first matmul


# ----------------------------------------------------------------------------
# host-side planning
# ----------------------------------------------------------------------------

def _sections(vl):
    out = []
    for v in vl:
        a = v // WBIG
        rem = v - a * WBIG
        nb_ = (rem + WSM - 1) // WSM
        if nb_ * WSM >= WBIG:
            a += 1
            nb_ = 0
        out.append((a, nb_))
    return out


def _plan(valid_lens):
    """Returns (na, nbm, cores); cores[c] = unit list (batch, row0, width),
    batch None for padding, ordered [small, big*na, small*(nbm-1)]."""
    vl = [int(v) for v in valid_lens]
    ab = _sections(vl)
    A = sum(a for a, _ in ab)
    Bs = sum(b for _, b in ab)
    best = None
    for na in range((A + NCORES - 1) // NCORES, -1, -1):
        d = max(0, A - NCORES * na)
        btot = Bs + 4 * d
        nbm = (btot + NCORES - 1) // NCORES
        cost = NCORES * (WBIG * na + WSM * nbm)
        if cost >= WBIG * A + WSM * Bs:
            if best is None or cost < best[0]:
                best = (cost, na, nbm, d)
    _, na, nbm, d = best
    ab = list(ab)
    for b in sorted(range(B), key=lambda x: -ab[x][0]):
        if d == 0:
            break
        a, s = ab[b]
        if a > 0:
            ab[b] = (a - 1, s + 4)
            d -= 1
    big_secs, sm_secs = [], []
    for b in range(B):
        a, s = ab[b]
        for i in range(a):
            big_secs.append((b, i * WBIG))
        for i in range(s):
            sm_secs.append((b, a * WBIG + i * WSM))
    cores = []
    bi = si = 0
    for c in range(NCORES):
        bigs, smalls = [], []
        for _ in range(na):
            bigs.append((*big_secs[bi], WBIG) if bi < len(big_secs)
                        else (None, 0, WBIG))
            bi += 1
        for _ in range(nbm):
            smalls.append((*sm_secs[si], WSM) if si < len(sm_secs)
                          else (None, 0, WSM))
            si += 1
        # optionally one small unit first (fast first matmul), then bigs,
        # then the rest; default big-first (matches v1's proven DMA overlap)
        if smalls and SMALL_FIRST:
            cores.append([smalls[0]] + bigs + smalls[1:])
        else:
            cores.append(bigs + smalls)
    return na, nbm, cores


def _widths(na, nbm):
    if nbm > 0 and SMALL_FIRST:
        return [WSM] + [WBIG] * na + [WSM] * (nbm - 1)
    return [WBIG] * na + [WSM] * nbm


def _core_arrays(units, keysT16, vb16, q32, vl):
    rows = sum(w for _, _, w in units)
    nu = len(units)
    qT = np.zeros((P, rows), dtype=np.float16)
    kT = np.zeros((P, nu, K), dtype=np.float16)
    vb = np.zeros((P, nu, NKT, D + 1), dtype=np.float16)
    col = 0
    for i, (b, r0, w) in enumerate(units):
        if b is not None:
            kT[:, i, :] = keysT16[b]
            vb[:, i, :, :] = vb16[b]
            nr = max(0, min(w, vl[b] - r0))
            if nr > 0:
                qT[:, col:col + nr] = q32[b, r0:r0 + nr, :].T
        else:
            vb[:, i, :, D] = 1.0      # keep denominators nonzero on padding
        col += w
    # header = [unit0 k-tiles 0-2 | full qT] so one DMA covers the critical
    # first-chunk inputs
    hd = np.concatenate([kT[:, 0, 0:3 * P], qT], axis=1)
    return {"hd": np.ascontiguousarray(hd), "kt": kT, "vb": vb}


# ----------------------------------------------------------------------------
# bass program (shared across cores; depends only on (na, nbm))
# ----------------------------------------------------------------------------

def _chunks_for(w, last=False):
    """(n_ktiles, engine) chunk list; 's' = ScalarE exact exp,
    'v' = DVE Schraudolph.  Scalar chunks are kept large (the 352-cycle
    ACTIVATE pipeline fill is NOT hidden between instructions, so act
    count is expensive); ktiles < SPLIT land in e1, >= SPLIT in e2
    regardless of producer.  The last unit is scalar-only so DVE is free
    to run the final normalize immediately."""
    if w == WBIG:
        return [(3, "s"), (3, "s"), (3, "s"), (3, "s"), (2, "v"), (2, "v")]
    if last:
        return [(8, "s"), (4, "s"), (4, "s")]
    return [(8, "s"), (4, "s"), (4, "v")]


def _build_program(na, nbm):
    nc = bacc.Bacc(name=f"attn_w_{na}_{nbm}")

    widths = _widths(na, nbm)
    nu = len(widths)
    rows = sum(widths)
    totqs = sum(w // P for w in widths)

    # hd = [unit0 k-tiles 0-2 | full qT]: the whole critical head lands in
    # one DMA (each dma_start pays ~1us of serialized DGE startup, so the
    # first matmul is gated by DMA COUNT ahead of it, not bytes)
    HKT = 3                      # unit-0 k-tiles packed into the header
    hd_d = nc.dram_tensor("hd", [P, HKT * P + rows], F16,
                          kind="ExternalInput")
    kt_d = nc.dram_tensor("kt", [P, nu, K], F16, kind="ExternalInput")
    vb_d = nc.dram_tensor("vb", [P, nu, NKT, D + 1], F16, kind="ExternalInput")
    out_d = nc.dram_tensor("out", [totqs, P, D], F16, kind="ExternalOutput")

    with tile.TileContext(nc) as tc, ExitStack() as ctx:
        sing = ctx.enter_context(tc.tile_pool(name="sing", bufs=1))
        e1pool = ctx.enter_context(tc.tile_pool(name="e1pool", bufs=3))
        e2pool = ctx.enter_context(tc.tile_pool(name="e2pool", bufs=3))
        opool = ctx.enter_context(tc.tile_pool(name="opool", bufs=4))
        rpool = ctx.enter_context(tc.tile_pool(name="rpool", bufs=4))
        ps_s = ctx.enter_context(tc.tile_pool(name="ps_s", bufs=2, space="PSUM"))
        ps_pv = ctx.enter_context(tc.tile_pool(name="ps_pv", bufs=2, space="PSUM"))

        HKT = 3
        kt_sb = sing.tile([P, nu, K], F16)
        vb_sb = sing.tile([P, nu, NKT, D + 1], F16)
        hd_sb = sing.tile([P, HKT * P + rows], F16)

        def qt_ap(c0, c1):                   # qT view inside the header tile
            return hd_sb[:, HKT * P + c0:HKT * P + c1]

        # PE warmup: dependency-free matmuls on zeroed scratch, run during
        # the DMA head so the HAM ramp is paid on garbage work.
        if NWARM:
            ws = sing.tile([P, P], F16)
            nc.gpsimd.memset(ws[:, :], 0)
            ps_w = ps_s.tile([P, 3, WBIG], F32, tag="ps", name="ps_warm")
            for _ in range(NWARM):
                nc.tensor.matmul(ps_w[:, 0, 0:P], lhsT=ws[:, :], rhs=ws[:, :])

        # head: one DMA carries unit-0's first 3 k-tiles plus its qT, so
        # the first S chunk is gated by a single transfer; then unit-0's
        # remaining k-tiles, the rest of qT, and per-unit kt/vb.
        w0 = widths[0]
        nc.sync.dma_start(out=hd_sb[:, 0:HKT * P + w0],
                          in_=hd_d[:, 0:HKT * P + w0])
        nc.sync.dma_start(out=kt_sb[:, 0, HKT * P:K],
                          in_=kt_d[:, 0, HKT * P:K])
        if rows > w0:
            nc.sync.dma_start(out=hd_sb[:, HKT * P + w0:],
                              in_=hd_d[:, HKT * P + w0:])
        nc.sync.dma_start(out=vb_sb[:, 0, :, :], in_=vb_d[:, 0, :, :])
        for u, w in list(enumerate(widths))[1:]:
            nc.sync.dma_start(out=kt_sb[:, u, :], in_=kt_d[:, u, :])
            nc.sync.dma_start(out=vb_sb[:, u, :, :], in_=vb_d[:, u, :, :])

        # pending PV drains: {e1, e2, u, nqs, oqs, j}
        pending = []

        def pv_lhs(rec, kt, j):
            if kt < SPLIT:
                return rec["e1"][:, kt, j * P:(j + 1) * P]
            return rec["e2"][:, kt - SPLIT, j * P:(j + 1) * P].bitcast(F16)

        def emit_pv_step():
            if not pending:
                return
            rec = pending[0]
            j = rec["j"]
            u = rec["u"]
            pv = ps_pv.tile([P, D + 1], F32, tag="pv")
            for kt in range(NKT):
                nc.tensor.matmul(
                    pv,
                    lhsT=pv_lhs(rec, kt, j),
                    rhs=vb_sb[:, u, kt, :],
                    start=(kt == 0),
                    stop=(kt == NKT - 1),
                )
            recip = rpool.tile([P, 1], F32, tag="recip")
            nc.vector.reciprocal(recip, pv[:, D:D + 1])
            o_sb = opool.tile([P, D], F16, tag="o")
            nc.vector.tensor_scalar_mul(o_sb, in0=pv[:, 0:D], scalar1=recip)
            nc.gpsimd.dma_start(out=out_d[rec["oqs"] + j, :, :], in_=o_sb)
            rec["j"] += 1
            if rec["j"] == rec["nqs"]:
                pending.pop(0)

        col = 0
        oqs = 0
        last_pv = None
        for u, w in enumerate(widths):
            nqs = w // P
            last = (u == nu - 1) and nqs == 1
            e1 = e1pool.tile([P, SPLIT, w], F16, tag="e1", name=f"e1_{u}")
            e2 = e2pool.tile([P, NKT - SPLIT, w], I16, tag="e2", name=f"e2_{u}")
            kt0 = 0
            for ci, (chn, eng) in enumerate(_chunks_for(w, last)):
                ps = ps_s.tile([P, chn, w], F32, tag="ps", name=f"ps{u}_{kt0}")
                for j in range(chn):
                    kt = kt0 + j
                    if u == 0 and kt < 3:    # unit-0 head k-tiles live in hd
                        lhsT = hd_sb[:, kt * P:(kt + 1) * P]
                    else:
                        lhsT = kt_sb[:, u, kt * P:(kt + 1) * P]
                    nc.tensor.matmul(
                        ps[:, j, :],
                        lhsT=lhsT,
                        rhs=qt_ap(col, col + w),
                    )
                if eng == "s":
                    if kt0 < SPLIT:
                        e_out = e1[:, kt0:kt0 + chn, :]
                    else:
                        e_out = e2[:, kt0 - SPLIT:kt0 - SPLIT + chn, :]
                        e_out = e_out.bitcast(F16)
                    nc.scalar.activation(
                        out=e_out,
                        in_=ps,
                        func=mybir.ActivationFunctionType.Exp,
                        scale=SCALE,
                    )
                else:
                    nc.vector.tensor_scalar(
                        out=e2[:, kt0 - SPLIT:kt0 - SPLIT + chn, :],
                        in0=ps,
                        scalar1=EXPA,
                        scalar2=EXPB,
                        op0=mybir.AluOpType.mult,
                        op1=mybir.AluOpType.add,
                    )
                kt0 += chn
                emit_pv_step()
                if last:
                    # drain the last unit's PV chunk-by-chunk as each act
                    # lands so the tail after the final chunk is short
                    if last_pv is None:
                        last_pv = ps_pv.tile([P, D + 1], F32, tag="pv")
                    for kt in range(kt0 - chn, kt0):
                        lhsT = (e1[:, kt, 0:P] if kt < SPLIT
                                else e2[:, kt - SPLIT, 0:P].bitcast(F16))
                        nc.tensor.matmul(
                            last_pv,
                            lhsT=lhsT,
                            rhs=vb_sb[:, u, kt, :],
                            start=(kt == 0),
                            stop=(kt == NKT - 1),
                        )
            if last:
                recip = rpool.tile([P, 1], F32, tag="recip")
                nc.vector.reciprocal(recip, last_pv[:, D:D + 1])
                o_sb = opool.tile([P, D], F16, tag="o")
                nc.vector.tensor_scalar_mul(
                    o_sb, in0=last_pv[:, 0:D], scalar1=recip)
                nc.sync.dma_start(out=out_d[oqs, :, :], in_=o_sb)
            else:
                pending.append(
                    {"e1": e1, "e2": e2, "u": u, "nqs": nqs, "oqs": oqs,
                     "j": 0})
            oqs += nqs
            col += w
        while pending:
            emit_pv_step()
    nc.compile()
    return nc


_NC_CACHE = {}


def _get_nc(na, nbm):
    key = (na, nbm)
    if key not in _NC_CACHE:
        _NC_CACHE[key] = _build_program(*key)
    return _NC_CACHE[key]


# ----------------------------------------------------------------------------
# top-level kernel
# ----------------------------------------------------------------------------

def _run(inputs: dict, trace: bool = False):
    q32 = np.ascontiguousarray(np.asarray(inputs["queries"], dtype=np.float32))
    k32 = np.ascontiguousarray(np.asarray(inputs["keys"], dtype=np.float32))
    v32 = np.ascontiguousarray(np.asarray(inputs["values"], dtype=np.float32))
    vl = np.asarray(inputs["valid_lens"]).astype(np.int64)

    if int(vl.sum()) == 0:           # every row masked: output is mean(V)
        meanv = v32.mean(axis=1)
        return np.broadcast_to(meanv[:, None, :], (B, Q, D)).copy(), None

    na, nbm, cores = _plan(vl)
    nc = _get_nc(na, nbm)

    keysT16 = np.ascontiguousarray(
        k32.transpose(0, 2, 1).astype(np.float16))          # [B,128,K]
    vb16 = np.ones((B, P, NKT, D + 1), dtype=np.float16)
    vb16[:, :, :, :D] = (
        v32.reshape(B, NKT, P, D).transpose(0, 2, 1, 3).astype(np.float16))

    in_maps = [
        _core_arrays(units, keysT16, vb16, q32, vl) for units in cores
    ]
    res = run_bass_kernel_spmd(
        nc, in_maps, core_ids=list(range(NCORES)), trace=trace)

    meanv = v32.mean(axis=1)                                 # [B, D]
    out = np.broadcast_to(meanv[:, None, :], (B, Q, D)).copy()
    for c, units in enumerate(cores):
        dev = res.results[c]["out"].astype(np.float32)       # [totqs,128,128]
        qs = 0
        for b, r0, w in units:
            for j in range(w // P):
                if b is not None:
                    lo = r0 + j * P
                    hi = min(int(vl[b]), lo + P)
                    if hi > lo:
                        out[b, lo:hi, :] = dev[qs, 0:hi - lo, :]
                qs += 1
    return out, res


def kernel(**inputs) -> np.ndarray:
    out, _ = _run(inputs, trace=False)
    return out
